# revision 61
# baseline (speedup 1.0000x reference)
"""BuddyPool kernel for Trainium2 (Bass/Tile), 8-core data-parallel.

Problem: cue (64,5,1024), patches (64,32,32,1024) ->
  sim = einsum('bkd,bhwd->bkhw'); idx = argmax(sim over hw);
  roi = mean of boundary-clamped 3x3 patch window around idx  -> (64,5,1024)

Sharding: batch across 8 cores, 8 samples/core. Inside one core:
  - samples 0..6 stream d-major ([128 hw-part, 8 c, 128 d] chunks): PE
    transposes -> pt copies -> sim matmuls pipeline inside the load, exactly
    the baseline schedule (gapless DMA at the 1456ns/chunk roofline)
  - sample 7 streams hw-block-major (8 blocks of [128 hw, 1024 d], 4KB
    descriptors, same transfer time): sim[:, block] finalizes per block, so
    a cheap per-block Max (258ns) + running elementwise max replace most of
    the final argmax; after the last block only Max+MaxIndex-full remain on
    the critical path
  - 3x3 window -> wgtT [hw, K] built by PE matmuls against precomputed
    row/col membership tables (rtab/ctab), replacing the DVE outer product
    + 8 transposes of the baseline
  - roi written in raw roiT [d-part, dc*K] layout (host un-transposes);
    all out DMAs deferred past the last load so the load stream stays
    gapless; the drain is the only serial tail
"""

import sys

if "/opt/trn_rl_repo" not in sys.path:
    sys.path.insert(0, "/opt/trn_rl_repo")

import numpy as np

import concourse.bass as bass
import concourse.tile as tile
from concourse import mybir
from concourse.masks import make_identity

P = 128
B = 64          # full batch
NCORES = 8
NS = B // NCORES  # samples per core
K = 5
D = 1024
H = W = 32
HW = H * W      # 1024
NDC = D // P    # 8 d-chunks
NHWC = HW // P  # 8 hw-chunks
F32 = mybir.dt.float32
F32R = mybir.dt.float32r
U32 = mybir.dt.uint32

TRANS_DT = F32R   # nat/pt tiles + transpose PSUM dtype
IDENT_DT = F32R   # walrus rejects bf16 ident x f32r data (32/non-32 mix)
LAG_D = 2         # sim matmuls lag transposes by this many d-chunks
NAT_BUFS = 4
PT_BUFS = 4
MAX_WAITS = 1
WAIT_DCS = 3      # build-time scheduling: delay sim matmuls by N chunk-times
WAIT_PER_S = 12500  # build-time scheduling: per-sample slope for the delay
LOAD_END = 95.98  # ms-hint just past the final load chunk's transfer

ALU = mybir.AluOpType


def split_multiwait_ctrl(nc, max_waits=1):
    """Walrus (neuronxcc CoreV3) rejects instructions carrying more than
    one sync wait. Hoist excess waits onto same-engine NOPs emitted just
    before the instruction — program order on the engine's sequencer makes
    this semantically identical (waits are a conjunction)."""
    n_split = 0
    for fn in nc.m.functions:
        for bb in fn.blocks:
            new_list = []
            for inst in bb.instructions:
                si = inst.sync_info
                lim = 1 if isinstance(inst, mybir.InstMatmult) else max_waits
                if si is not None and si.on_wait and len(si.on_wait) > lim:
                    waits = list(si.on_wait)
                    extra, keep = waits[:-lim], waits[-lim:]
                    for i, w in enumerate(extra):
                        d = mybir.InstNoOp(
                            name=f"{inst.name}-ws{i}",
                            engine=inst.engine,
                            ins=[],
                            outs=[],
                            sync_info=mybir.SyncInfo(on_wait=[w], on_update=[]),
                        )
                        nc.register_instruction(d)
                        new_list.append(d)
                    si.on_wait = keep
                    n_split += 1
                new_list.append(inst)
            bb.instructions[:] = new_list
    return n_split


def build_bass():
    nc = bass.Bass(
        trn_type="TRN2",
        target_bir_lowering=False,
        debug=False,
        enable_asserts=False,
    )

    cue_d = nc.dram_tensor("cue", [NS * K, D], F32, kind="ExternalInput").ap()
    pat_d = nc.dram_tensor("patches", [NS * HW, D], F32, kind="ExternalInput").ap()
    # raw roiT layout: out2[p, s*40 + dc*5 + k] = roi[s][k, dc*128+p]
    out2_d = nc.dram_tensor(
        "out2", [P, NS * NDC * K], F32, kind="ExternalOutput"
    ).ap()

    with tile.TileContext(nc) as tc:
        build_kernel(tc, out2_d, cue_d, pat_d)
    split_multiwait_ctrl(nc, max_waits=MAX_WAITS)
    return nc


def build_kernel(tc, out2_d, cue_d, pat_d):
    nc = tc.nc
    from contextlib import ExitStack

    ctx = ExitStack()
    const = ctx.enter_context(tc.tile_pool(name="const", bufs=1))
    natp = ctx.enter_context(tc.tile_pool(name="nat", bufs=NAT_BUFS))
    ptp = ctx.enter_context(tc.tile_pool(name="pt", bufs=PT_BUFS))
    smallp = ctx.enter_context(tc.tile_pool(name="small", bufs=2))
    pst = ctx.enter_context(tc.tile_pool(name="ps_t", bufs=3, space="PSUM"))
    pss = ctx.enter_context(tc.tile_pool(name="ps_s", bufs=2, space="PSUM"))
    psrT = ctx.enter_context(tc.tile_pool(name="ps_rt", bufs=1, space="PSUM"))

    # ---- sample 0 patch loads ahead of everything (SP queue = loads only) --
    nat_tiles = {}

    def issue_loads(s):
        nat = natp.tile([P, NHWC, D], TRANS_DT, tag="nat")
        nat_tiles[s] = nat
        src = pat_d[s * HW : (s + 1) * HW, :].rearrange(
            "(c p) d -> p c d", p=P
        ).bitcast(TRANS_DT)
        if s < NS - 1:
            # d-major: [128 hw-part, 8 c, 128 d] slices, 512B descriptors
            for dc in range(NDC):
                nc.sync.dma_start(
                    out=nat[:, :, dc * P : (dc + 1) * P],
                    in_=src[:, :, dc * P : (dc + 1) * P],
                )
        else:
            # hw-block-major: [128 hw, 1024 d] per block, 4KB descriptors.
            # Same SBUF tile layout — only the fill order changes. The last
            # block is split in d-halves so its first-half transposes/copies
            # start one half-transfer earlier (the block feeds the drain).
            for c in range(NHWC):
                if c < NHWC - 1:
                    nc.sync.dma_start(out=nat[:, c, :], in_=src[:, c, :])
                else:
                    nc.sync.dma_start(
                        out=nat[:, c, 0 : D // 2], in_=src[:, c, 0 : D // 2]
                    )
                    nc.sync.dma_start(
                        out=nat[:, c, D // 2 : D], in_=src[:, c, D // 2 : D]
                    )
        return nat

    cue_sb = const.tile([NS * K, D], F32)
    issue_loads(0)

    # ---- constants ----
    ident_f = const.tile([P, P], F32)
    make_identity(nc, ident_f[:])
    ident = const.tile([P, P], IDENT_DT)
    nc.vector.tensor_copy(out=ident[:], in_=ident_f[:])

    # iota pair [K, 2, 32]: row 0 = 0..31 (h grid), row 1 = 0..31 (w grid)
    io_u = const.tile([K, 2, 32], U32)
    nc.gpsimd.iota(io_u[:], pattern=[[0, 2], [1, 32]], channel_multiplier=0)
    io2 = const.tile([K, 2, 32], F32)
    nc.vector.tensor_copy(out=io2[:], in_=io_u[:])

    # window tables for the wgtT matmuls: the ±1 window AND the 1/span
    # normalization live here, so the per-sample DVE chain only builds a
    # one-hot of (h, w):
    #   rtab[h, c, p] = (|(c*128+p)//32 - h| <= 1) / span(h)   [32, 8, 128]
    #   ctab[w, p]    = (|p%32 - w| <= 1) / span(w)            [32, 128]
    rg_u = const.tile([32, NHWC * P], U32)
    nc.gpsimd.iota(rg_u[:], pattern=[[1, 32], [0, 32]], channel_multiplier=0)
    hc_u = const.tile([32, 1], U32)
    nc.gpsimd.iota(hc_u[:], pattern=[[0, 1]], channel_multiplier=1)
    pm_u = const.tile([32, P], U32)
    nc.gpsimd.iota(pm_u[:], pattern=[[0, 4], [1, 32]], channel_multiplier=0)
    rtab = const.tile([32, NHWC, P], F32)
    ctab = const.tile([32, P], F32)
    hc_f = const.tile([32, 1], F32)
    inv_f = const.tile([32, 2], F32)
    tabtmp = const.tile([32, NHWC * P], F32)
    nc.vector.tensor_copy(out=hc_f[:], in_=hc_u[:])
    # inv[h] = 1/3 + (1/6)*(h==0 | h==31)  (exact for H=W=32)
    nc.vector.tensor_scalar(
        out=inv_f[:, 0:1], in0=hc_f[:], scalar1=0.0, scalar2=None,
        op0=ALU.is_equal,
    )
    nc.vector.tensor_scalar(
        out=inv_f[:, 1:2], in0=hc_f[:], scalar1=float(H - 1), scalar2=None,
        op0=ALU.is_equal,
    )
    nc.vector.tensor_tensor(
        out=inv_f[:, 0:1], in0=inv_f[:, 0:1], in1=inv_f[:, 1:2], op=ALU.add
    )
    nc.vector.tensor_scalar(
        out=inv_f[:, 0:1], in0=inv_f[:, 0:1], scalar1=1.0 / 6.0,
        scalar2=1.0 / 3.0, op0=ALU.mult, op1=ALU.add,
    )
    rtab_flat = rtab[:].rearrange("h c p -> h (c p)")
    nc.vector.tensor_copy(out=rtab_flat, in_=rg_u[:])
    nc.vector.tensor_tensor(
        out=rtab_flat,
        in0=rtab_flat,
        in1=hc_f[:].broadcast_to((32, NHWC * P)),
        op=ALU.subtract,
    )
    nc.vector.tensor_scalar(
        out=tabtmp[:], in0=rtab_flat, scalar1=-1.0, scalar2=1.0,
        op0=ALU.max, op1=ALU.min,
    )
    nc.vector.tensor_tensor(
        out=rtab_flat, in0=tabtmp[:], in1=rtab_flat, op=ALU.is_equal
    )
    nc.vector.tensor_tensor(
        out=rtab_flat,
        in0=rtab_flat,
        in1=inv_f[:, 0:1].broadcast_to((32, NHWC * P)),
        op=ALU.mult,
    )
    nc.vector.tensor_copy(out=ctab[:], in_=pm_u[:])
    nc.vector.tensor_tensor(
        out=ctab[:],
        in0=ctab[:],
        in1=hc_f[:].broadcast_to((32, P)),
        op=ALU.subtract,
    )
    nc.vector.tensor_scalar(
        out=tabtmp[:, 0:P], in0=ctab[:], scalar1=-1.0, scalar2=1.0,
        op0=ALU.max, op1=ALU.min,
    )
    nc.vector.tensor_tensor(
        out=ctab[:], in0=tabtmp[:, 0:P], in1=ctab[:], op=ALU.is_equal
    )
    nc.vector.tensor_tensor(
        out=ctab[:],
        in0=ctab[:],
        in1=inv_f[:, 0:1].broadcast_to((32, P)),
        op=ALU.mult,
    )

    # roi accumulator in raw roiT layout, written per sample, DMA'd at the end
    roiT_all = const.tile([P, NS, NDC * K], F32)

    # ---- cue -> cueT ----
    nc.scalar.dma_start(out=cue_sb[:], in_=cue_d[:])
    ident_cue = const.tile([NS * K, NS * K], F32)
    make_identity(nc, ident_cue[:])
    cueT = const.tile([P, NDC, NS * K], TRANS_DT)
    for dc in range(NDC):
        ps = pst.tile([P, 512], F32, tag="pst")
        nc.tensor.transpose(
            out=ps[:, : NS * K],
            in_=cue_sb[:, dc * P : (dc + 1) * P],
            identity=ident_cue[:],
        )
        nc.vector.tensor_copy(out=cueT[:, dc, :], in_=ps[:, : NS * K])

    # ------------------------------------------------------------------
    # Per-sample stages. wgtT/roiT of sample s-1 are interleaved into
    # sample s's front stream at fixed points so no engine queue
    # head-of-line blocks on the serial argmax chain.
    # ------------------------------------------------------------------
    state = {}  # s -> dict with sim_ps etc.

    def masks_tail(s, idx8):
        """idx -> h/w -> one-hot m2 [K, 2, 32] (row 0: h, row 1: w)."""
        st = state[s]
        sc = smallp.tile([K, 8], F32, tag="sc")
        hw_u = smallp.tile([K, 2], U32, tag="hwu")
        nc.vector.tensor_scalar(
            out=hw_u[:, 0:1], in0=idx8[:, 0:1], scalar1=5, scalar2=None,
            op0=ALU.logical_shift_right,
        )
        nc.vector.tensor_scalar(
            out=hw_u[:, 1:2], in0=idx8[:, 0:1], scalar1=31, scalar2=None,
            op0=ALU.bitwise_and,
        )
        nc.vector.tensor_copy(out=sc[:, 0:2], in_=hw_u[:])

        # one-hot of (h, w): the window and normalization live in rtab/ctab
        m2 = smallp.tile([K, 2, 32], F32, tag="m2")
        nc.vector.tensor_tensor(
            out=m2[:],
            in0=io2[:],
            in1=sc[:, 0:2].unsqueeze(2).broadcast_to((K, 2, 32)),
            op=ALU.is_equal,
        )
        st["m2"] = m2

    def stage_wgtT(s):
        """m2 [K, 2, 32] -> wgtT [128, c, K] via 2 PE transposes + one ACT
        copy + 9 table matmuls + 1 DVE multiply. Replaces the baseline's
        DVE outer product + 8 transposes: much shorter serial chain."""
        st = state[s]
        m2f = st["m2"][:].rearrange("k a b -> k (a b)")
        psT_t = pst.tile([P, 512], F32, tag="pst", name="mskT")
        psT = psT_t[:32, :]
        nc.tensor.transpose(
            out=psT[:, 0:K], in_=m2f[:, 0:32], identity=ident_cue[:K, :K]
        )
        nc.tensor.transpose(
            out=psT[:, K : 2 * K], in_=m2f[:, 32:64],
            identity=ident_cue[:K, :K],
        )
        msk = smallp.tile([32, 2, K], F32, tag="mskT")
        if s == NS - 1:
            nc.vector.tensor_copy(out=msk[:], in_=psT[:, 0 : 2 * K])
        else:
            nc.scalar.copy(out=msk[:], in_=psT[:, 0 : 2 * K])
        psw_t = psrT.tile([P, 64], F32, tag="roiT", name="wgtmm")
        psw = psw_t[:]
        for c in range(NHWC):
            nc.tensor.matmul(
                out=psw[:, c * K : (c + 1) * K],
                lhsT=rtab[:, c, :],
                rhs=msk[:, 0, :],
                skip_group_check=True,
            )
        nc.tensor.matmul(
            out=psw[:, NHWC * K : NHWC * K + K],
            lhsT=ctab[:],
            rhs=msk[:, 1, :],
            skip_group_check=True,
        )
        # rm -> SBUF via one ACT copy; the DVE multiply then reads cm from
        # PSUM (a single PSUM operand is legal; two are not — 1 read port)
        rm_sb = smallp.tile([P, NHWC, K], F32, tag="rmsb")
        if s == NS - 1:
            nc.vector.tensor_copy(
                out=rm_sb[:].rearrange("p c k -> p (c k)"),
                in_=psw[:, 0 : NHWC * K],
            )
        else:
            nc.scalar.copy(
                out=rm_sb[:].rearrange("p c k -> p (c k)"),
                in_=psw[:, 0 : NHWC * K],
            )
        wgtT = smallp.tile([P, NHWC, K], F32, tag="wgtT")
        nc.vector.tensor_tensor(
            out=wgtT[:],
            in0=rm_sb[:],
            in1=psw[:, NHWC * K : NHWC * K + K].unsqueeze(1).broadcast_to(
                (P, NHWC, K)
            ),
            op=ALU.mult,
        )
        st["wgtT"] = wgtT

    def stage_roiT(s):
        """roiT[d, k] = sum_hw patches[hw, d] * wgt[k, hw], computed per
        (c, dc) with nat as the STATIONARY operand. Result copied into the
        resident roiT_all accumulator (raw layout, no final transposes)."""
        st = state[s]
        nat = nat_tiles[s]
        wgtT = st["wgtT"]
        roiT_ps_t = psrT.tile([P, 64], F32, tag="roiT", name="roimm")
        roiT_ps = roiT_ps_t[:, 0 : NDC * K]
        # fp32 (bitcast from f32r — same bits): HW rejects fp32r matmuls
        # with a 5-wide moving operand (s3d3_mm_fp32r_restrictions).
        # dc OUTER: each dc's accumulation group must fully complete before
        # the next opens — interleaved groups within one PSUM bank corrupt.
        for dc in range(NDC):
            for c in range(NHWC):
                nc.tensor.matmul(
                    out=roiT_ps[:, dc * K : (dc + 1) * K],
                    lhsT=nat[:, c, dc * P : (dc + 1) * P].bitcast(F32),
                    rhs=wgtT[:, c, :],
                    start=(c == 0),
                    stop=(c == NHWC - 1),
                    skip_group_check=True,
                )
        if s == NS - 1:
            nc.vector.tensor_copy(out=roiT_all[:, s, :], in_=roiT_ps[:])
        else:
            nc.scalar.copy(out=roiT_all[:, s, :], in_=roiT_ps[:])

    def stage_argmax(s):
        """Monolithic argmax for d-major samples (hidden under the load)."""
        st = state[s]
        sim_ps = st["sim_ps"]
        mx8 = smallp.tile([K, 8], F32, tag="mx8")
        idx8 = smallp.tile([K, 8], U32, tag="idx8")
        nc.vector.max(out=mx8[:], in_=sim_ps[:])
        nc.vector.max_index(out=idx8[:], in_max=mx8[:], in_values=sim_ps[:])
        masks_tail(s, idx8)

    def stage_front(s, prev):
        """d-major front for samples 0..NS-2 (baseline schedule)."""
        nat = nat_tiles[s]
        sim_ps = pss.tile([K, HW], F32, tag="sim")
        state[s] = {"sim_ps": sim_ps}

        def sim_mms(dc, pt):
            # Schedule-order hint (build-time only): the Tile list scheduler
            # otherwise hoists each sim matmul into the transpose->copy
            # serial loop, putting its copy-wait on the PE critical cycle.
            t0 = 2300 + s * WAIT_PER_S
            with tc.tile_wait_until(
                (t0 + (dc + WAIT_DCS) * 1456) / 1e6,
                enable=WAIT_DCS > 0,
            ):
                for hf in range(2):
                    nc.tensor.matmul(
                        out=sim_ps[:, hf * 512 : (hf + 1) * 512],
                        lhsT=cueT[:, dc, s * K : (s + 1) * K],
                        rhs=pt[:, hf * 512 : (hf + 1) * 512],
                        start=(dc == 0),
                        stop=(dc == NDC - 1),
                        skip_group_check=True,
                    )

        pending = []
        for dc in range(NDC):
            pt = ptp.tile([P, HW], TRANS_DT, tag="pt")
            for hf in range(2):
                ps = pst.tile([P, 512], TRANS_DT, tag="pst")
                for q in range(4):
                    c = hf * 4 + q
                    nc.tensor.matmul(
                        out=ps[:, q * P : (q + 1) * P],
                        lhsT=nat[:, c, dc * P : (dc + 1) * P],
                        rhs=ident[:],
                        is_transpose=True,
                        skip_group_check=True,
                    )
                nc.scalar.copy(out=pt[:, hf * 512 : (hf + 1) * 512], in_=ps[:])
            pending.append((dc, pt))
            if len(pending) > LAG_D:
                sim_mms(*pending.pop(0))
        for item in pending:
            sim_mms(*item)
        # Sample prev's roi work rides the PE slack AFTER this sample's sim
        # stream (never in the middle: its DVE/ACT gates would head-of-line
        # block the stream).
        if prev is not None:
            stage_wgtT(prev)
            stage_roiT(prev)
        # argmax chain for THIS sample: issued at the stream tail so the
        # DVE ops start the moment the last sim matmul lands. For the
        # penultimate sample, hint it past s7's early-block chains (its
        # 2.4us of DVE otherwise lands exactly when blocks 0-1 need DVE).
        with tc.tile_wait_until(91.0e3 / 1e6, enable=s == NS - 2):
            stage_argmax(s)

    def stage_front_last(prev):
        """hw-block-major front for the last sample: per-block transposes ->
        copies -> 8 accumulating simT matmuls (ptb stationary, cue moving:
        ~8ns/mm) -> DVE copy -> PE transpose back into sim[K, hw] ->
        per-block Max merged into a running max. After the final block only
        MaxIndex-full + masks + the wgtT/roiT chain remain: the drain."""
        s = NS - 1
        nat = nat_tiles[s]
        sim_ps = pss.tile([K, HW], F32, tag="sim")
        state[s] = {"sim_ps": sim_ps}
        mxr = smallp.tile([K, 8], F32, tag="mxr")

        # end of s7 block 0's transfer in the gapless stream (head 2300 +
        # cue 455 + 57 chunks); per-block hints pin each block's work at its
        # data-arrival point so the scheduler interleaves it with s6's
        # (late-hinted) sim stream instead of queueing it after
        T0 = 2300 + 455 + ((NS - 1) * NDC + 1) * 1456

        for c in range(NHWC):
            tc.tile_set_cur_wait((T0 + c * 1456) / 1e6)
            ptb = ptp.tile([P, HW], TRANS_DT, tag="pt")
            # simT[hw, k] accumulated with ptb STATIONARY and cue MOVING
            # (5-wide, f32-bitcast): ~8ns/mm vs 213ns for the 128-wide f32r
            # form (<256 moving cols costs 4x in the PE). Allocated from the
            # psrT ring (PSUM banks are full; the ring serializes vs the
            # wgtT/roiT tiles, which are short-lived).
            simT_t = psrT.tile([P, 64], F32, tag="roiT", name="simT")
            simT = simT_t[:, 0:K]
            for hf in range(2):
                ps = pst.tile([P, 512], TRANS_DT, tag="pst")
                for q in range(4):
                    dc = hf * 4 + q
                    nc.tensor.matmul(
                        out=ps[:, q * P : (q + 1) * P],
                        lhsT=nat[:, c, dc * P : (dc + 1) * P],
                        rhs=ident[:],
                        is_transpose=True,
                        skip_group_check=True,
                    )
                dst = ptb[:, hf * 512 : (hf + 1) * 512]
                # balanced copy split: the last two blocks' h1 go to DVE so
                # they overlap ACT's h0 right at drain entry; ACT takes the
                # rest
                if hf == 1 and c >= NHWC - 2:
                    nc.vector.tensor_copy(out=dst, in_=ps[:])
                else:
                    nc.scalar.copy(out=dst, in_=ps[:])
                for q in range(4):
                    dc = hf * 4 + q
                    nc.tensor.matmul(
                        out=simT,
                        lhsT=ptb[:, dc * P : (dc + 1) * P].bitcast(F32),
                        rhs=cueT[:, dc, s * K : (s + 1) * K].bitcast(F32),
                        start=(dc == 0),
                        stop=(dc == NDC - 1),
                        skip_group_check=True,
                    )
            simT_sb = smallp.tile([P, K], F32, tag="simTsb")
            nc.vector.tensor_copy(out=simT_sb[:], in_=simT)
            nc.tensor.transpose(
                out=sim_ps[:, c * P : (c + 1) * P],
                in_=simT_sb[:],
                identity=ident_f[:],
            )
            mxb = smallp.tile([K, 8], F32, tag="mxb")
            nc.vector.max(out=mxb[:], in_=sim_ps[:, c * P : (c + 1) * P])
            if c == 0:
                nc.vector.tensor_copy(out=mxr[:], in_=mxb[:])
            else:
                nc.vector.tensor_tensor(
                    out=mxr[:], in0=mxr[:], in1=mxb[:], op=ALU.max
                )
            # prev's roi work rides early-block slack (all engines are
            # loosely loaded here; keeps the drain window clear)
            if c == 1 and prev is not None:
                stage_wgtT(prev)
                stage_roiT(prev)

        # ---- drain chain ----
        idx8 = smallp.tile([K, 8], U32, tag="idx8")
        nc.vector.max_index(out=idx8[:], in_max=mxr[:], in_values=sim_ps[:])
        masks_tail(s, idx8)
        stage_wgtT(s)
        stage_roiT(s)

    # ---- pipeline across samples ----
    for s in range(NS):
        if s + 1 < NS:
            issue_loads(s + 1)
        if s < NS - 1:
            stage_front(s, s - 1 if s > 0 else None)
        else:
            stage_front_last(s - 1)

    # ---- deferred output DMAs (SP HWDGE, pinned past the last load so
    # their transfers never preempt a DMA_ENGINES slot mid-stream) ----
    with tc.tile_wait_until(LOAD_END):
        nc.sync.dma_start(
            out=out2_d[:, 0 : (NS - 1) * NDC * K],
            in_=roiT_all[:, 0 : NS - 1, :],
        )
    with tc.tile_wait_until(LOAD_END + 0.0001):
        nc.sync.dma_start(
            out=out2_d[:, (NS - 1) * NDC * K : NS * NDC * K],
            in_=roiT_all[:, NS - 1, :],
        )

    ctx.close()


def make_in_maps(cue, patches):
    cue = np.ascontiguousarray(np.asarray(cue, np.float32)).reshape(B, K, D)
    patches = np.ascontiguousarray(np.asarray(patches, np.float32)).reshape(
        B, HW, D
    )
    in_maps = []
    for c in range(NCORES):
        in_maps.append(
            {
                "cue": np.ascontiguousarray(
                    cue[c * NS : (c + 1) * NS].reshape(NS * K, D)
                ),
                "patches": np.ascontiguousarray(
                    patches[c * NS : (c + 1) * NS].reshape(NS * HW, D)
                ),
            }
        )
    return in_maps


_NC_CACHE = None


def get_nc():
    global _NC_CACHE
    if _NC_CACHE is None:
        _NC_CACHE = build_bass()
    return _NC_CACHE


def run(cue, patches, trace=False):
    from concourse.bass_utils import run_bass_kernel_spmd

    nc = get_nc()
    in_maps = make_in_maps(cue, patches)
    res = run_bass_kernel_spmd(
        nc, in_maps, core_ids=list(range(NCORES)), trace=trace
    )
    outs = []
    for r in res.results:
        o2 = np.asarray(r["out2"], np.float32)  # [P, NS*NDC*K]
        o = o2.reshape(P, NS, NDC, K).transpose(1, 3, 2, 0).reshape(NS, K, D)
        outs.append(o)
    full = np.concatenate(outs, axis=0)
    return full, res


def kernel(cue, patches):
    full, _ = run(cue, patches, trace=False)
    return full


# revision 63
# speedup vs baseline: 1.0007x; 1.0007x over previous
"""BuddyPool kernel for Trainium2 (Bass/Tile), 8-core data-parallel.

Problem: cue (64,5,1024), patches (64,32,32,1024) ->
  sim = einsum('bkd,bhwd->bkhw'); idx = argmax(sim over hw);
  roi = mean of boundary-clamped 3x3 patch window around idx  -> (64,5,1024)

Sharding: batch across 8 cores, 8 samples/core. Inside one core:
  - samples 0..6 stream d-major ([128 hw-part, 8 c, 128 d] chunks): PE
    transposes -> pt copies -> sim matmuls pipeline inside the load, exactly
    the baseline schedule (gapless DMA at the 1456ns/chunk roofline)
  - sample 7 streams hw-block-major (8 blocks of [128 hw, 1024 d], 4KB
    descriptors, same transfer time): sim[:, block] finalizes per block, so
    a cheap per-block Max (258ns) + running elementwise max replace most of
    the final argmax; after the last block only Max+MaxIndex-full remain on
    the critical path
  - 3x3 window -> wgtT [hw, K] built by PE matmuls against precomputed
    row/col membership tables (rtab/ctab), replacing the DVE outer product
    + 8 transposes of the baseline
  - roi written in raw roiT [d-part, dc*K] layout (host un-transposes);
    all out DMAs deferred past the last load so the load stream stays
    gapless; the drain is the only serial tail
"""

import sys

if "/opt/trn_rl_repo" not in sys.path:
    sys.path.insert(0, "/opt/trn_rl_repo")

import numpy as np

import concourse.bass as bass
import concourse.tile as tile
from concourse import mybir
from concourse.masks import make_identity

P = 128
B = 64          # full batch
NCORES = 8
NS = B // NCORES  # samples per core
K = 5
D = 1024
H = W = 32
HW = H * W      # 1024
NDC = D // P    # 8 d-chunks
NHWC = HW // P  # 8 hw-chunks
F32 = mybir.dt.float32
F32R = mybir.dt.float32r
U32 = mybir.dt.uint32

TRANS_DT = F32R   # nat/pt tiles + transpose PSUM dtype
IDENT_DT = F32R   # walrus rejects bf16 ident x f32r data (32/non-32 mix)
LAG_D = 2         # sim matmuls lag transposes by this many d-chunks
NAT_BUFS = 4
PT_BUFS = 4
MAX_WAITS = 1
WAIT_DCS = 3      # build-time scheduling: delay sim matmuls by N chunk-times
WAIT_PER_S = 12500  # build-time scheduling: per-sample slope for the delay
LOAD_END = 95.98  # ms-hint just past the final load chunk's transfer

ALU = mybir.AluOpType


def split_multiwait_ctrl(nc, max_waits=1):
    """Walrus (neuronxcc CoreV3) rejects instructions carrying more than
    one sync wait. Hoist excess waits onto same-engine NOPs emitted just
    before the instruction — program order on the engine's sequencer makes
    this semantically identical (waits are a conjunction)."""
    n_split = 0
    for fn in nc.m.functions:
        for bb in fn.blocks:
            new_list = []
            for inst in bb.instructions:
                si = inst.sync_info
                lim = 1 if isinstance(inst, mybir.InstMatmult) else max_waits
                if si is not None and si.on_wait and len(si.on_wait) > lim:
                    waits = list(si.on_wait)
                    extra, keep = waits[:-lim], waits[-lim:]
                    for i, w in enumerate(extra):
                        d = mybir.InstNoOp(
                            name=f"{inst.name}-ws{i}",
                            engine=inst.engine,
                            ins=[],
                            outs=[],
                            sync_info=mybir.SyncInfo(on_wait=[w], on_update=[]),
                        )
                        nc.register_instruction(d)
                        new_list.append(d)
                    si.on_wait = keep
                    n_split += 1
                new_list.append(inst)
            bb.instructions[:] = new_list
    return n_split


def build_bass():
    nc = bass.Bass(
        trn_type="TRN2",
        target_bir_lowering=False,
        debug=False,
        enable_asserts=False,
    )

    cue_d = nc.dram_tensor("cue", [NS * K, D], F32, kind="ExternalInput").ap()
    pat_d = nc.dram_tensor("patches", [NS * HW, D], F32, kind="ExternalInput").ap()
    # raw roiT layout: out2[p, s*40 + dc*5 + k] = roi[s][k, dc*128+p]
    out2_d = nc.dram_tensor(
        "out2", [P, NS * NDC * K], F32, kind="ExternalOutput"
    ).ap()

    with tile.TileContext(nc) as tc:
        build_kernel(tc, out2_d, cue_d, pat_d)
    split_multiwait_ctrl(nc, max_waits=MAX_WAITS)
    return nc


def build_kernel(tc, out2_d, cue_d, pat_d):
    nc = tc.nc
    from contextlib import ExitStack

    ctx = ExitStack()
    const = ctx.enter_context(tc.tile_pool(name="const", bufs=1))
    natp = ctx.enter_context(tc.tile_pool(name="nat", bufs=NAT_BUFS))
    ptp = ctx.enter_context(tc.tile_pool(name="pt", bufs=PT_BUFS))
    smallp = ctx.enter_context(tc.tile_pool(name="small", bufs=2))
    pst = ctx.enter_context(tc.tile_pool(name="ps_t", bufs=3, space="PSUM"))
    pss = ctx.enter_context(tc.tile_pool(name="ps_s", bufs=2, space="PSUM"))
    psrT = ctx.enter_context(tc.tile_pool(name="ps_rt", bufs=1, space="PSUM"))

    # ---- sample 0 patch loads ahead of everything (SP queue = loads only) --
    nat_tiles = {}

    def issue_loads(s):
        nat = natp.tile([P, NHWC, D], TRANS_DT, tag="nat")
        nat_tiles[s] = nat
        src = pat_d[s * HW : (s + 1) * HW, :].rearrange(
            "(c p) d -> p c d", p=P
        ).bitcast(TRANS_DT)
        if s < NS - 1:
            # d-major: [128 hw-part, 8 c, 128 d] slices, 512B descriptors
            for dc in range(NDC):
                nc.sync.dma_start(
                    out=nat[:, :, dc * P : (dc + 1) * P],
                    in_=src[:, :, dc * P : (dc + 1) * P],
                )
        else:
            # hw-block-major: [128 hw, 1024 d] per block, 4KB descriptors.
            # Same SBUF tile layout — only the fill order changes. The last
            # block is split in d-halves so its first-half transposes/copies
            # start one half-transfer earlier (the block feeds the drain).
            for c in range(NHWC):
                if c < NHWC - 1:
                    nc.sync.dma_start(out=nat[:, c, :], in_=src[:, c, :])
                else:
                    nc.sync.dma_start(
                        out=nat[:, c, 0 : D // 2], in_=src[:, c, 0 : D // 2]
                    )
                    nc.sync.dma_start(
                        out=nat[:, c, D // 2 : D], in_=src[:, c, D // 2 : D]
                    )
        return nat

    cue_sb = const.tile([NS * K, D], F32)
    issue_loads(0)

    # ---- constants ----
    ident_f = const.tile([P, P], F32)
    make_identity(nc, ident_f[:])
    ident = const.tile([P, P], IDENT_DT)
    nc.vector.tensor_copy(out=ident[:], in_=ident_f[:])

    # iota pair [K, 2, 32]: row 0 = 0..31 (h grid), row 1 = 0..31 (w grid)
    io_u = const.tile([K, 2, 32], U32)
    nc.gpsimd.iota(io_u[:], pattern=[[0, 2], [1, 32]], channel_multiplier=0)
    io2 = const.tile([K, 2, 32], F32)
    nc.vector.tensor_copy(out=io2[:], in_=io_u[:])

    # window tables for the wgtT matmuls: the ±1 window AND the 1/span
    # normalization live here, so the per-sample DVE chain only builds a
    # one-hot of (h, w):
    #   rtab[h, c, p] = (|(c*128+p)//32 - h| <= 1) / span(h)   [32, 8, 128]
    #   ctab[w, p]    = (|p%32 - w| <= 1) / span(w)            [32, 128]
    rg_u = const.tile([32, NHWC * P], U32)
    nc.gpsimd.iota(rg_u[:], pattern=[[1, 32], [0, 32]], channel_multiplier=0)
    hc_u = const.tile([32, 1], U32)
    nc.gpsimd.iota(hc_u[:], pattern=[[0, 1]], channel_multiplier=1)
    pm_u = const.tile([32, P], U32)
    nc.gpsimd.iota(pm_u[:], pattern=[[0, 4], [1, 32]], channel_multiplier=0)
    rtab = const.tile([32, NHWC, P], F32)
    ctab = const.tile([32, P], F32)
    hc_f = const.tile([32, 1], F32)
    inv_f = const.tile([32, 2], F32)
    tabtmp = const.tile([32, NHWC * P], F32)
    nc.vector.tensor_copy(out=hc_f[:], in_=hc_u[:])
    # inv[h] = 1/3 + (1/6)*(h==0 | h==31)  (exact for H=W=32)
    nc.vector.tensor_scalar(
        out=inv_f[:, 0:1], in0=hc_f[:], scalar1=0.0, scalar2=None,
        op0=ALU.is_equal,
    )
    nc.vector.tensor_scalar(
        out=inv_f[:, 1:2], in0=hc_f[:], scalar1=float(H - 1), scalar2=None,
        op0=ALU.is_equal,
    )
    nc.vector.tensor_tensor(
        out=inv_f[:, 0:1], in0=inv_f[:, 0:1], in1=inv_f[:, 1:2], op=ALU.add
    )
    nc.vector.tensor_scalar(
        out=inv_f[:, 0:1], in0=inv_f[:, 0:1], scalar1=1.0 / 6.0,
        scalar2=1.0 / 3.0, op0=ALU.mult, op1=ALU.add,
    )
    rtab_flat = rtab[:].rearrange("h c p -> h (c p)")
    nc.vector.tensor_copy(out=rtab_flat, in_=rg_u[:])
    nc.vector.tensor_tensor(
        out=rtab_flat,
        in0=rtab_flat,
        in1=hc_f[:].broadcast_to((32, NHWC * P)),
        op=ALU.subtract,
    )
    nc.vector.tensor_scalar(
        out=tabtmp[:], in0=rtab_flat, scalar1=-1.0, scalar2=1.0,
        op0=ALU.max, op1=ALU.min,
    )
    nc.vector.tensor_tensor(
        out=rtab_flat, in0=tabtmp[:], in1=rtab_flat, op=ALU.is_equal
    )
    nc.vector.tensor_tensor(
        out=rtab_flat,
        in0=rtab_flat,
        in1=inv_f[:, 0:1].broadcast_to((32, NHWC * P)),
        op=ALU.mult,
    )
    nc.vector.tensor_copy(out=ctab[:], in_=pm_u[:])
    nc.vector.tensor_tensor(
        out=ctab[:],
        in0=ctab[:],
        in1=hc_f[:].broadcast_to((32, P)),
        op=ALU.subtract,
    )
    nc.vector.tensor_scalar(
        out=tabtmp[:, 0:P], in0=ctab[:], scalar1=-1.0, scalar2=1.0,
        op0=ALU.max, op1=ALU.min,
    )
    nc.vector.tensor_tensor(
        out=ctab[:], in0=tabtmp[:, 0:P], in1=ctab[:], op=ALU.is_equal
    )
    nc.vector.tensor_tensor(
        out=ctab[:],
        in0=ctab[:],
        in1=inv_f[:, 0:1].broadcast_to((32, P)),
        op=ALU.mult,
    )

    # roi accumulator in raw roiT layout, written per sample, DMA'd at the end
    roiT_all = const.tile([P, NS, NDC * K], F32)

    # ---- cue -> cueT ----
    nc.scalar.dma_start(out=cue_sb[:], in_=cue_d[:])
    ident_cue = const.tile([NS * K, NS * K], F32)
    make_identity(nc, ident_cue[:])
    cueT = const.tile([P, NDC, NS * K], TRANS_DT)
    for dc in range(NDC):
        ps = pst.tile([P, 512], F32, tag="pst")
        nc.tensor.transpose(
            out=ps[:, : NS * K],
            in_=cue_sb[:, dc * P : (dc + 1) * P],
            identity=ident_cue[:],
        )
        nc.vector.tensor_copy(out=cueT[:, dc, :], in_=ps[:, : NS * K])

    # ------------------------------------------------------------------
    # Per-sample stages. wgtT/roiT of sample s-1 are interleaved into
    # sample s's front stream at fixed points so no engine queue
    # head-of-line blocks on the serial argmax chain.
    # ------------------------------------------------------------------
    state = {}  # s -> dict with sim_ps etc.

    def masks_tail(s, idx8):
        """idx -> h/w -> one-hot m2 [K, 2, 32] (row 0: h, row 1: w)."""
        st = state[s]
        sc = smallp.tile([K, 8], F32, tag="sc")
        hw_u = smallp.tile([K, 2], U32, tag="hwu")
        nc.vector.tensor_scalar(
            out=hw_u[:, 0:1], in0=idx8[:, 0:1], scalar1=5, scalar2=None,
            op0=ALU.logical_shift_right,
        )
        nc.vector.tensor_scalar(
            out=hw_u[:, 1:2], in0=idx8[:, 0:1], scalar1=31, scalar2=None,
            op0=ALU.bitwise_and,
        )
        # one-hot of (h, w): the window and normalization live in rtab/ctab.
        # u32 compare with f32 output skips the u32->f32 convert stage.
        m2 = smallp.tile([K, 2, 32], F32, tag="m2")
        nc.vector.tensor_tensor(
            out=m2[:],
            in0=io_u[:],
            in1=hw_u[:].unsqueeze(2).broadcast_to((K, 2, 32)),
            op=ALU.is_equal,
        )
        st["m2"] = m2

    def stage_wgtT(s):
        """m2 [K, 2, 32] -> wgtT [128, c, K] via 2 PE transposes + one ACT
        copy + 9 table matmuls + 1 DVE multiply. Replaces the baseline's
        DVE outer product + 8 transposes: much shorter serial chain."""
        st = state[s]
        m2f = st["m2"][:].rearrange("k a b -> k (a b)")
        psT_t = pst.tile([P, 512], F32, tag="pst", name="mskT")
        psT = psT_t[:32, :]
        nc.tensor.transpose(
            out=psT[:, 0:K], in_=m2f[:, 0:32], identity=ident_cue[:K, :K]
        )
        nc.tensor.transpose(
            out=psT[:, K : 2 * K], in_=m2f[:, 32:64],
            identity=ident_cue[:K, :K],
        )
        msk = smallp.tile([32, 2, K], F32, tag="mskT")
        if s == NS - 1:
            nc.vector.tensor_copy(out=msk[:], in_=psT[:, 0 : 2 * K])
        else:
            nc.scalar.copy(out=msk[:], in_=psT[:, 0 : 2 * K])
        psw_t = psrT.tile([P, 64], F32, tag="roiT", name="wgtmm")
        psw = psw_t[:]
        for c in range(NHWC):
            nc.tensor.matmul(
                out=psw[:, c * K : (c + 1) * K],
                lhsT=rtab[:, c, :],
                rhs=msk[:, 0, :],
                skip_group_check=True,
            )
        nc.tensor.matmul(
            out=psw[:, NHWC * K : NHWC * K + K],
            lhsT=ctab[:],
            rhs=msk[:, 1, :],
            skip_group_check=True,
        )
        # rm -> SBUF via one ACT copy; the DVE multiply then reads cm from
        # PSUM (a single PSUM operand is legal; two are not — 1 read port)
        rm_sb = smallp.tile([P, NHWC, K], F32, tag="rmsb")
        if s == NS - 1:
            nc.vector.tensor_copy(
                out=rm_sb[:].rearrange("p c k -> p (c k)"),
                in_=psw[:, 0 : NHWC * K],
            )
        else:
            nc.scalar.copy(
                out=rm_sb[:].rearrange("p c k -> p (c k)"),
                in_=psw[:, 0 : NHWC * K],
            )
        wgtT = smallp.tile([P, NHWC, K], F32, tag="wgtT")
        nc.vector.tensor_tensor(
            out=wgtT[:],
            in0=rm_sb[:],
            in1=psw[:, NHWC * K : NHWC * K + K].unsqueeze(1).broadcast_to(
                (P, NHWC, K)
            ),
            op=ALU.mult,
        )
        st["wgtT"] = wgtT

    def stage_roiT(s):
        """roiT[d, k] = sum_hw patches[hw, d] * wgt[k, hw], computed per
        (c, dc) with nat as the STATIONARY operand. Result copied into the
        resident roiT_all accumulator (raw layout, no final transposes)."""
        st = state[s]
        nat = nat_tiles[s]
        wgtT = st["wgtT"]
        roiT_ps_t = psrT.tile([P, 64], F32, tag="roiT", name="roimm")
        roiT_ps = roiT_ps_t[:, 0 : NDC * K]
        # fp32 (bitcast from f32r — same bits): HW rejects fp32r matmuls
        # with a 5-wide moving operand (s3d3_mm_fp32r_restrictions).
        # dc OUTER: each dc's accumulation group must fully complete before
        # the next opens — interleaved groups within one PSUM bank corrupt.
        for dc in range(NDC):
            for c in range(NHWC):
                nc.tensor.matmul(
                    out=roiT_ps[:, dc * K : (dc + 1) * K],
                    lhsT=nat[:, c, dc * P : (dc + 1) * P].bitcast(F32),
                    rhs=wgtT[:, c, :],
                    start=(c == 0),
                    stop=(c == NHWC - 1),
                    skip_group_check=True,
                )
        if s == NS - 1:
            nc.vector.tensor_copy(out=roiT_all[:, s, :], in_=roiT_ps[:])
        else:
            nc.scalar.copy(out=roiT_all[:, s, :], in_=roiT_ps[:])

    def stage_argmax(s):
        """Monolithic argmax for d-major samples (hidden under the load)."""
        st = state[s]
        sim_ps = st["sim_ps"]
        mx8 = smallp.tile([K, 8], F32, tag="mx8")
        idx8 = smallp.tile([K, 8], U32, tag="idx8")
        nc.vector.max(out=mx8[:], in_=sim_ps[:])
        nc.vector.max_index(out=idx8[:], in_max=mx8[:], in_values=sim_ps[:])
        masks_tail(s, idx8)

    def stage_front(s, prev):
        """d-major front for samples 0..NS-2 (baseline schedule)."""
        nat = nat_tiles[s]
        sim_ps = pss.tile([K, HW], F32, tag="sim")
        state[s] = {"sim_ps": sim_ps}

        def sim_mms(dc, pt):
            # Schedule-order hint (build-time only): the Tile list scheduler
            # otherwise hoists each sim matmul into the transpose->copy
            # serial loop, putting its copy-wait on the PE critical cycle.
            t0 = 2300 + s * WAIT_PER_S
            with tc.tile_wait_until(
                (t0 + (dc + WAIT_DCS) * 1456) / 1e6,
                enable=WAIT_DCS > 0,
            ):
                for hf in range(2):
                    nc.tensor.matmul(
                        out=sim_ps[:, hf * 512 : (hf + 1) * 512],
                        lhsT=cueT[:, dc, s * K : (s + 1) * K],
                        rhs=pt[:, hf * 512 : (hf + 1) * 512],
                        start=(dc == 0),
                        stop=(dc == NDC - 1),
                        skip_group_check=True,
                    )

        pending = []
        for dc in range(NDC):
            pt = ptp.tile([P, HW], TRANS_DT, tag="pt")
            for hf in range(2):
                ps = pst.tile([P, 512], TRANS_DT, tag="pst")
                for q in range(4):
                    c = hf * 4 + q
                    nc.tensor.matmul(
                        out=ps[:, q * P : (q + 1) * P],
                        lhsT=nat[:, c, dc * P : (dc + 1) * P],
                        rhs=ident[:],
                        is_transpose=True,
                        skip_group_check=True,
                    )
                nc.scalar.copy(out=pt[:, hf * 512 : (hf + 1) * 512], in_=ps[:])
            pending.append((dc, pt))
            if len(pending) > LAG_D:
                sim_mms(*pending.pop(0))
        for item in pending:
            sim_mms(*item)
        # Sample prev's roi work rides the PE slack AFTER this sample's sim
        # stream (never in the middle: its DVE/ACT gates would head-of-line
        # block the stream).
        if prev is not None:
            stage_wgtT(prev)
            stage_roiT(prev)
        # argmax chain for THIS sample: issued at the stream tail so the
        # DVE ops start the moment the last sim matmul lands
        stage_argmax(s)

    def stage_front_last(prev):
        """hw-block-major front for the last sample: per-block transposes ->
        copies -> 8 accumulating simT matmuls (ptb stationary, cue moving:
        ~8ns/mm) -> DVE copy -> PE transpose back into sim[K, hw] ->
        per-block Max merged into a running max. After the final block only
        MaxIndex-full + masks + the wgtT/roiT chain remain: the drain."""
        s = NS - 1
        nat = nat_tiles[s]
        sim_ps = pss.tile([K, HW], F32, tag="sim")
        state[s] = {"sim_ps": sim_ps}
        mxr = smallp.tile([K, 8], F32, tag="mxr")

        # end of s7 block 0's transfer in the gapless stream (head 2300 +
        # cue 455 + 57 chunks); per-block hints pin each block's work at its
        # data-arrival point so the scheduler interleaves it with s6's
        # (late-hinted) sim stream instead of queueing it after
        T0 = 2300 + 455 + ((NS - 1) * NDC + 1) * 1456

        for c in range(NHWC):
            tc.tile_set_cur_wait((T0 + c * 1456) / 1e6)
            ptb = ptp.tile([P, HW], TRANS_DT, tag="pt")
            # simT[hw, k] accumulated with ptb STATIONARY and cue MOVING
            # (5-wide, f32-bitcast): ~8ns/mm vs 213ns for the 128-wide f32r
            # form (<256 moving cols costs 4x in the PE). Allocated from the
            # psrT ring (PSUM banks are full; the ring serializes vs the
            # wgtT/roiT tiles, which are short-lived).
            simT_t = psrT.tile([P, 64], F32, tag="roiT", name="simT")
            simT = simT_t[:, 0:K]
            for hf in range(2):
                ps = pst.tile([P, 512], TRANS_DT, tag="pst")
                for q in range(4):
                    dc = hf * 4 + q
                    nc.tensor.matmul(
                        out=ps[:, q * P : (q + 1) * P],
                        lhsT=nat[:, c, dc * P : (dc + 1) * P],
                        rhs=ident[:],
                        is_transpose=True,
                        skip_group_check=True,
                    )
                dst = ptb[:, hf * 512 : (hf + 1) * 512]
                # balanced copy split: the last two blocks' h1 go to DVE so
                # they overlap ACT's h0 right at drain entry; ACT takes the
                # rest
                if hf == 1 and c >= NHWC - 2:
                    nc.vector.tensor_copy(out=dst, in_=ps[:])
                else:
                    nc.scalar.copy(out=dst, in_=ps[:])
                for q in range(4):
                    dc = hf * 4 + q
                    nc.tensor.matmul(
                        out=simT,
                        lhsT=ptb[:, dc * P : (dc + 1) * P].bitcast(F32),
                        rhs=cueT[:, dc, s * K : (s + 1) * K].bitcast(F32),
                        start=(dc == 0),
                        stop=(dc == NDC - 1),
                        skip_group_check=True,
                    )
            simT_sb = smallp.tile([P, K], F32, tag="simTsb")
            nc.vector.tensor_copy(out=simT_sb[:], in_=simT)
            nc.tensor.transpose(
                out=sim_ps[:, c * P : (c + 1) * P],
                in_=simT_sb[:],
                identity=ident_f[:],
            )
            mxb = smallp.tile([K, 8], F32, tag="mxb")
            nc.vector.max(out=mxb[:], in_=sim_ps[:, c * P : (c + 1) * P])
            if c == 0:
                nc.vector.tensor_copy(out=mxr[:], in_=mxb[:])
            else:
                nc.vector.tensor_tensor(
                    out=mxr[:], in0=mxr[:], in1=mxb[:], op=ALU.max
                )
            # prev's roi work rides early-block slack (all engines are
            # loosely loaded here; keeps the drain window clear)
            if c == 1 and prev is not None:
                stage_wgtT(prev)
                stage_roiT(prev)

        # ---- drain chain ----
        idx8 = smallp.tile([K, 8], U32, tag="idx8")
        nc.vector.max_index(out=idx8[:], in_max=mxr[:], in_values=sim_ps[:])
        masks_tail(s, idx8)
        stage_wgtT(s)
        stage_roiT(s)

    # ---- pipeline across samples ----
    for s in range(NS):
        if s + 1 < NS:
            issue_loads(s + 1)
        if s < NS - 1:
            stage_front(s, s - 1 if s > 0 else None)
        else:
            stage_front_last(s - 1)

    # ---- deferred output DMAs (SP HWDGE, pinned past the last load so
    # their transfers never preempt a DMA_ENGINES slot mid-stream) ----
    with tc.tile_wait_until(LOAD_END):
        nc.sync.dma_start(
            out=out2_d[:, 0 : (NS - 1) * NDC * K],
            in_=roiT_all[:, 0 : NS - 1, :],
        )
    with tc.tile_wait_until(LOAD_END + 0.0001):
        nc.sync.dma_start(
            out=out2_d[:, (NS - 1) * NDC * K : NS * NDC * K],
            in_=roiT_all[:, NS - 1, :],
        )

    ctx.close()


def make_in_maps(cue, patches):
    cue = np.ascontiguousarray(np.asarray(cue, np.float32)).reshape(B, K, D)
    patches = np.ascontiguousarray(np.asarray(patches, np.float32)).reshape(
        B, HW, D
    )
    in_maps = []
    for c in range(NCORES):
        in_maps.append(
            {
                "cue": np.ascontiguousarray(
                    cue[c * NS : (c + 1) * NS].reshape(NS * K, D)
                ),
                "patches": np.ascontiguousarray(
                    patches[c * NS : (c + 1) * NS].reshape(NS * HW, D)
                ),
            }
        )
    return in_maps


_NC_CACHE = None


def get_nc():
    global _NC_CACHE
    if _NC_CACHE is None:
        _NC_CACHE = build_bass()
    return _NC_CACHE


def run(cue, patches, trace=False):
    from concourse.bass_utils import run_bass_kernel_spmd

    nc = get_nc()
    in_maps = make_in_maps(cue, patches)
    res = run_bass_kernel_spmd(
        nc, in_maps, core_ids=list(range(NCORES)), trace=trace
    )
    outs = []
    for r in res.results:
        o2 = np.asarray(r["out2"], np.float32)  # [P, NS*NDC*K]
        o = o2.reshape(P, NS, NDC, K).transpose(1, 3, 2, 0).reshape(NS, K, D)
        outs.append(o)
    full = np.concatenate(outs, axis=0)
    return full, res


def kernel(cue, patches):
    full, _ = run(cue, patches, trace=False)
    return full
